# revision 59
# baseline (speedup 1.0000x reference)
"""Trainium2 Bass kernel for nn_NewsEntityGNN (2-layer GraphSAGE + BatchNorm).

Math (per reference):
  h  = relu(BN0(mean_agg(x) @ W_l0 + x @ W_r0))      # biases drop out under BN
  out = BN1(mean_agg(h) @ W_l1 + h @ W_r1)
  BN uses batch statistics over all 50000 nodes (biased var), eps=1e-5.

Distribution: nodes are range-partitioned across 8 NeuronCores (6250 each).
Each core aggregates the edges whose destination it owns:
  - edges grouped on host by 128-node destination blocks, split by source
    range (lo: src<32768 / hi: src>=32768 to satisfy int16 gather indices),
    padded to 128-edge tiles; tile counts equalized across cores so one SPMD
    program serves all 8 cores (per-core shortfall is padded with dstloc=-1
    lanes that contribute nothing).
  - gathers run on 4 SWDGE queues (4 parallel Q7 descgen pairs, ~2.5ns/idx)
    in 1024-index chunks, issued eagerly with deep buffering so the gpsimd
    engine never stalls on consumers.
  - per tile: a one-hot matrix S[128 edges, 128 dst] built on DVE
    (iota + is_equal) scatters the gathered rows via one fp16 TensorE matmul
    into a PSUM accumulator [128 dst, 128 feat]. In-degree reciprocals are
    precomputed on host (they depend only on edge_index).
  - per 128-dst block: scale by 1/max(cnt,1), PE-transpose, two matmuls with
    the (replicated) weight matrices, BatchNorm stats via ACT accumulators.
  - cross-core: AllReduce for BN statistics, AllGather for the layer-0
    output table that layer 1 gathers from.
"""

import os

import numpy as np

import concourse.bass as bass
import concourse.bacc as bacc
import concourse.tile as tile
from concourse import mybir
from concourse.bass_utils import run_bass_kernel_spmd

# problem shapes (hardcoded per contract)
N_NODES = 50000
N_EDGES = 800000
IN_DIM = 100
HID = 128
EPS = 1e-5

NC = 8
NPC = N_NODES // NC          # 6250 nodes per core
P = 128
NBLK = (NPC + P - 1) // P    # 49 dst blocks (= windows) per core
RSPLIT = 4096                # local-row split: seg0 rows [0,4096), seg1 rest
SEG_LEN = [RSPLIT, NPC - RSPLIT]
SEG_ROWS = [NC * SEG_LEN[0], NC * SEG_LEN[1]]   # 25600 / 24400 (< 32768)
D = 128                      # padded feature dim
TPC = 8                      # tiles per gather chunk (1024 tokens)
NTOK = TPC * P
NQ = 4                       # SWDGE queues round-robined across gathers
GBUFS = 12                   # in-flight gather chunk buffers per stream
AG_SPLITS = [0, 13, 26, 39, NBLK]  # AllGather sub-ranges (block granularity)

f16 = mybir.dt.float16
f32 = mybir.dt.float32
i16 = mybir.dt.int16


# ---------------------------------------------------------------- host prep

def _build_schedule(edge_index):
    """Group edges by (core, dst-block, src-half); equalize tile counts across
    cores. Returns the common schedule plus per-core gather/dstloc arrays and
    host-computed inverse in-degree."""
    src = np.asarray(edge_index[0], dtype=np.int64)
    dst = np.asarray(edge_index[1], dtype=np.int64)
    core = dst // NPC
    dloc = dst % NPC
    blk = dloc // P
    src_core = src // NPC
    src_row = src % NPC
    half = (src_row >= RSPLIT).astype(np.int64)
    # index within the segment table [NC, seg_len, D] (ranks-major)
    seg_idx = np.where(
        half == 0,
        src_core * SEG_LEN[0] + src_row,
        src_core * SEG_LEN[1] + (src_row - RSPLIT))
    key = (core * NBLK + blk) * 2 + half
    order = np.argsort(key, kind="stable")
    s_src = seg_idx[order]
    s_dloc = dloc[order]
    counts = np.bincount(key, minlength=NC * NBLK * 2).reshape(NC, NBLK, 2)
    starts = np.zeros(NC * NBLK * 2 + 1, dtype=np.int64)
    np.cumsum(counts.reshape(-1), out=starts[1:])

    tiles = -(-counts // P)                      # ceil
    T = tiles.max(axis=0)                        # [NBLK, 2] tiles per group
    T[:, 0] = np.maximum(T[:, 0], 1)             # psum coverage guarantee

    # processing tile sequence (common to all cores)
    proc = []                                    # (stream, stream_pos, block)
    spos = [0, 0]
    for b in range(NBLK):
        for h in (0, 1):
            for _ in range(int(T[b, h])):
                proc.append((h, spos[h], b))
                spos[h] += 1
    n_tiles = [spos[0], spos[1]]                 # lo/hi stream tile counts
    n_chunks = [-(-n_tiles[0] // TPC), -(-n_tiles[1] // TPC)]

    base_of = [np.concatenate([[0], np.cumsum(T[:, h])]).astype(np.int64)
               for h in (0, 1)]

    # in-degree reciprocal per destination (host-side; index-derived)
    deg = np.bincount(dst, minlength=N_NODES).astype(np.float64)
    invd = (1.0 / np.maximum(deg, 1.0)).astype(np.float32)

    # per-core slot arrays: srcs + dstloc per stream slot
    idx_arrs = [[], []]
    dl_arr = []
    invc_arr = []
    for c in range(NC):
        slot_src = [np.zeros(n_chunks[h] * NTOK, np.int64) for h in (0, 1)]
        slot_dl = [np.full(n_chunks[h] * NTOK, -1.0, np.float32) for h in (0, 1)]
        for b in range(NBLK):
            for h in (0, 1):
                g = (c * NBLK + b) * 2 + h
                cnt = counts[c, b, h]
                base = base_of[h][b]
                e0 = starts[g]
                sl = slice(base * P, base * P + cnt)
                slot_src[h][sl] = s_src[e0 : e0 + cnt]
                slot_dl[h][sl] = s_dloc[e0 : e0 + cnt] % P
        # wrapped+replicated int16 index layout per chunk
        for h in (0, 1):
            a = slot_src[h].astype(np.int16).reshape(n_chunks[h], NTOK // 16, 16)
            wr = a.transpose(0, 2, 1).reshape(n_chunks[h], 16, NTOK // 16)
            rep = np.tile(wr, (1, 8, 1)).transpose(1, 0, 2).reshape(P, -1)
            idx_arrs[h].append(np.ascontiguousarray(rep))
        # dstloc in processing-tile order [128, n_proc_tiles]
        dl = np.empty((P, len(proc)), np.float32)
        for j, (h, sp, _b) in enumerate(proc):
            dl[:, j] = slot_dl[h][sp * P : (sp + 1) * P]
        dl_arr.append(dl.astype(np.float16))
        # inverse in-degree replicated across partitions: iv[p, n] = 1/deg(n)
        own = invd[c * NPC : (c + 1) * NPC]
        iv = np.zeros((P, NBLK * P), np.float32)
        iv[:, :NPC] = own[None, :]
        invc_arr.append(iv)

    return dict(
        proc=proc, T=T, n_tiles=n_tiles, n_chunks=n_chunks,
        idx_lo=idx_arrs[0], idx_hi=idx_arrs[1],
        dstloc=dl_arr, invc=invc_arr,
    )


# ------------------------------------------------------------ device program

def _build_program(sched):
    proc = sched["proc"]
    n_chunks = sched["n_chunks"]
    NT = len(proc)
    SBATCH = 4                                   # S tiles built per DVE op

    nc = bacc.Bacc("TRN2", target_bir_lowering=False, num_swdge_queues=NQ)

    xs0 = nc.dram_tensor("xs0", [SEG_ROWS[0], D], f16, kind="ExternalInput")
    xs1 = nc.dram_tensor("xs1", [SEG_ROWS[1], D], f16, kind="ExternalInput")
    idx_lo = nc.dram_tensor("idx_lo", [P, n_chunks[0] * NTOK // 16], i16, kind="ExternalInput")
    idx_hi = nc.dram_tensor("idx_hi", [P, n_chunks[1] * NTOK // 16], i16, kind="ExternalInput")
    dstloc = nc.dram_tensor("dstloc", [P, NT], f16, kind="ExternalInput")
    xT_own = nc.dram_tensor("xT_own", [P, NPC], f16, kind="ExternalInput")
    iota128 = nc.dram_tensor("iota128", [P, P], f16, kind="ExternalInput")
    id16 = nc.dram_tensor("id16", [P, P], f16, kind="ExternalInput")
    id32 = nc.dram_tensor("id32", [P, P], f32, kind="ExternalInput")
    w_all = nc.dram_tensor("w_all", [P, 4 * P], f16, kind="ExternalInput")  # wl0|wr0|wl1|wr1
    gb = nc.dram_tensor("gb", [P, 4], f32, kind="ExternalInput")  # g0|b0|g1|b1
    invc_in = nc.dram_tensor("invc_in", [P, NBLK * P], f32, kind="ExternalInput")

    out_own = nc.dram_tensor("out_own", [NPC, HID], f32, kind="ExternalOutput")

    h0_own = nc.dram_tensor("h0_own", [NPC, D], f16)
    dbg_h0 = (nc.dram_tensor("dbg_h0", [NPC, D], f16, kind="ExternalOutput")
              if os.environ.get("DBG_H0") else None)
    dbg_hpre = (nc.dram_tensor("dbg_hpre", [P, NBLK * P], f32, kind="ExternalOutput")
                if os.environ.get("DBG_H0") else None)
    t0 = nc.dram_tensor("t0", [SEG_ROWS[0], D], f16, addr_space="Shared")
    t1 = nc.dram_tensor("t1", [SEG_ROWS[1], D], f16, addr_space="Shared")
    st_in = [nc.dram_tensor(f"st{l}_in", [P, 2], f32) for l in (0, 1)]
    st_out = [nc.dram_tensor(f"st{l}_out", [P, 2], f32, addr_space="Shared") for l in (0, 1)]

    NPC_PAD = NBLK * P

    with tile.TileContext(nc) as tc:
        with (
            tc.tile_pool(name="pers", bufs=1) as pers,
            tc.tile_pool(name="glo", bufs=GBUFS) as glo_pool,
            tc.tile_pool(name="ghi", bufs=GBUFS) as ghi_pool,
            tc.tile_pool(name="sb", bufs=6) as s_pool,
            tc.tile_pool(name="scr", bufs=2) as scr,
            tc.tile_pool(name="psA", bufs=4, space="PSUM") as psA,
            tc.tile_pool(name="psH", bufs=2, space="PSUM") as psH,
            tc.tile_pool(name="psB", bufs=2, space="PSUM") as psB,
        ):
            # ---- persistent loads ----
            ixl = pers.tile([P, n_chunks[0] * NTOK // 16], i16)
            ixh = pers.tile([P, n_chunks[1] * NTOK // 16], i16)
            dl = pers.tile([P, NT], f16)
            xT = pers.tile([P, NPC], f16)
            iota = pers.tile([P, P], f16)
            idT16 = pers.tile([P, P], f16)
            idT32 = pers.tile([P, P], f32)
            wt = pers.tile([P, 4 * P], f16)
            gbt = pers.tile([P, 4], f32)
            invc = pers.tile([P, NBLK * P], f32)
            eps_t = pers.tile([P, 1], f32)
            nc.vector.memset(eps_t[:], EPS)
            nc.sync.dma_start(out=ixl[:], in_=idx_lo[:])
            nc.sync.dma_start(out=ixh[:], in_=idx_hi[:])
            nc.sync.dma_start(out=dl[:], in_=dstloc[:])
            nc.sync.dma_start(out=xT[:], in_=xT_own[:])
            nc.sync.dma_start(out=iota[:], in_=iota128[:])
            nc.sync.dma_start(out=idT16[:], in_=id16[:])
            nc.sync.dma_start(out=idT32[:], in_=id32[:])
            nc.sync.dma_start(out=wt[:], in_=w_all[:])
            nc.sync.dma_start(out=gbt[:], in_=gb[:])
            nc.sync.dma_start(out=invc[:], in_=invc_in[:])

            hpre = pers.tile([P, NPC_PAD], f32)
            hT0 = pers.tile([P, NPC_PAD], f16)
            ssum = pers.tile([P, NBLK], f32)
            ssq = pers.tile([P, NBLK], f32)
            ntok_reg = nc.gpsimd.to_reg(NTOK)

            # chunk issue order = first-use order; per-chunk last-use tile
            issue_seq = []
            first_use = {}
            last_use = {}
            for j in range(NT):
                h, sp, _b = proc[j]
                ck = (h, sp // TPC)
                if ck not in first_use:
                    first_use[ck] = j
                    issue_seq.append(ck)
                last_use[ck] = j

            for layer in (0, 1):

                wl = wt[:, layer * 2 * P : layer * 2 * P + P]
                wr = wt[:, (layer * 2 + 1) * P : (layer * 2 + 2) * P]
                gamma = gbt[:, 2 * layer : 2 * layer + 1]
                beta = gbt[:, 2 * layer + 1 : 2 * layer + 2]

                # ---- phase A: aggregate + dense per block ----
                chunk_buf = [{}, {}]
                pools = [glo_pool, ghi_pool]
                tabs = ([xs0[:, :], xs1[:, :]] if layer == 0
                        else [t0[:, :], t1[:, :]])
                ixs = [ixl, ixh]
                s_bufs = {}
                qn = [0]

                def issue_chunk(h, k):
                    buf = pools[h].tile([P, TPC, D], f16, tag=f"g{h}")
                    nc.gpsimd.dma_gather(
                        out_ap=buf[:],
                        in_ap=tabs[h],
                        idxs_ap=ixs[h][:, k * NTOK // 16 : (k + 1) * NTOK // 16],
                        num_idxs=NTOK,
                        num_idxs_reg=ntok_reg,
                        elem_size=D,
                        queue_num=qn[0] % NQ,
                    )
                    qn[0] += 1
                    chunk_buf[h][k] = buf

                # Stagger the 4 SWDGE queues with dummy partial gathers so
                # chunk completions arrive steadily (~2.2us apart) instead of
                # in convoys of 4 every ~9us.
                for qi, nidx in ((1, 256), (2, 512), (3, 768)):
                    dbuf = scr.tile([P, nidx // P, D], f16, tag=f"stag{qi}")
                    nc.gpsimd.dma_gather(
                        out_ap=dbuf[:],
                        in_ap=tabs[0],
                        idxs_ap=ixs[0][:, 0 : nidx // 16],
                        num_idxs=nidx,
                        num_idxs_reg=nidx,
                        elem_size=D,
                        queue_num=qi,
                    )

                # Eager chunk issue with per-pool in-flight cap: a pool slot
                # may be re-written only after its previous chunk's consumers
                # are emitted, so keep issued-minus-retired < GBUFS per pool.
                # Layer 1 front-loads seg0 chunks: they depend only on the
                # first sub-AllGather, so they overlap the second one.
                layer_seq = list(issue_seq)
                if layer == 1:
                    head = [ck for ck in issue_seq if ck[0] == 0][: GBUFS - 1]
                    hs = set(head)
                    layer_seq = head + [ck for ck in issue_seq if ck not in hs]
                in_flight = [0, 0]

                def top_up(cur_tile):
                    while layer_seq:
                        h, k = layer_seq[0]
                        if in_flight[h] >= GBUFS - 1:
                            break
                        layer_seq.pop(0)
                        issue_chunk(h, k)
                        in_flight[h] += 1

                def retire(j):
                    h, sp, _b = proc[j]
                    ck = (h, sp // TPC)
                    if last_use[ck] == j:
                        in_flight[ck[0]] -= 1

                def get_sbatch(jb):
                    if jb not in s_bufs:
                        nb = min(SBATCH, NT - jb * SBATCH)
                        sb_t = s_pool.tile([P, SBATCH, P], f16, tag="S")
                        dsl = dl[:, jb * SBATCH : jb * SBATCH + nb]
                        dl_b = bass.AP(dl.tensor, dsl.offset, [dsl.ap[0], dsl.ap[1], [0, P]])
                        io_b = bass.AP(iota.tensor, iota[:].offset,
                                       [iota[:].ap[0], [0, nb], iota[:].ap[1]])
                        nc.vector.tensor_tensor(
                            out=sb_t[:, 0:nb, :], in0=io_b, in1=dl_b,
                            op=mybir.AluOpType.is_equal)
                        s_bufs[jb] = sb_t
                    return s_bufs[jb]

                def emit_evict(b, agg):
                    # agg holds aggT [feat, dst]; evict with per-dst 1/deg
                    aggT = scr.tile([P, P], f16, tag="aggT")
                    nc.vector.tensor_tensor(
                        out=aggT[:], in0=agg[:],
                        in1=invc[:, b * P : b * P + P],
                        op=mybir.AluOpType.mult)
                    return aggT

                def emit_post(b, aggT):
                    nb = min(P, NPC - b * P)
                    hps = psH.tile([P, P], f32, tag="h")
                    root = xT if layer == 0 else hT0
                    nc.tensor.matmul(out=hps[:, 0:nb], lhsT=wl[:, :],
                                     rhs=aggT[:, 0:nb], start=True, stop=False)
                    nc.tensor.matmul(out=hps[:, 0:nb], lhsT=wr[:, :],
                                     rhs=root[:, b * P : b * P + nb],
                                     start=False, stop=True)
                    nc.scalar.activation(
                        out=hpre[:, b * P : b * P + nb], in_=hps[:, 0:nb],
                        func=mybir.ActivationFunctionType.Copy,
                        accum_out=ssum[:, b : b + 1])
                    sqs = scr.tile([P, P], f32, tag="sq")
                    nc.scalar.activation(
                        out=sqs[:, 0:nb], in_=hps[:, 0:nb],
                        func=mybir.ActivationFunctionType.Square,
                        accum_out=ssq[:, b : b + 1])

                jidx = 0
                pending = None
                for b in range(NBLK):
                    agg = psA.tile([P, D], f32, tag="agg")
                    first = True
                    j0 = jidx
                    while jidx < NT and proc[jidx][2] == b:
                        h, sp, _b = proc[jidx]
                        top_up(jidx)
                        if sp // TPC not in chunk_buf[h]:
                            # cap-blocked in order walk; force the needed chunk
                            issue_chunk(h, sp // TPC)
                            in_flight[h] += 1
                            layer_seq.remove((h, sp // TPC))
                        g = chunk_buf[h][sp // TPC]
                        s_t = get_sbatch(jidx // SBATCH)
                        last = jidx + 1 >= NT or proc[jidx + 1][2] != b
                        nc.tensor.matmul(
                            out=agg[:, :],
                            lhsT=g[:, sp % TPC, :],
                            rhs=s_t[:, jidx % SBATCH, :],
                            start=first, stop=last)
                        first = False
                        retire(jidx)
                        jidx += 1
                    assert jidx > j0, f"block {b} has no tiles"
                    aggT = emit_evict(b, agg)
                    if pending is not None:
                        emit_post(*pending)
                    pending = (b, aggT)
                emit_post(*pending)
                chunk_buf[0].clear()
                chunk_buf[1].clear()
                s_bufs.clear()

                # ---- phase B: global BN stats ----
                if layer == 0 and dbg_hpre is not None:
                    nc.sync.dma_start(out=dbg_hpre[:, :], in_=hpre[:, :])
                stats = scr.tile([P, 2], f32, tag="stats")
                nc.vector.tensor_reduce(
                    out=stats[:, 0:1], in_=ssum[:, 0:NBLK],
                    op=mybir.AluOpType.add, axis=mybir.AxisListType.X)
                nc.vector.tensor_reduce(
                    out=stats[:, 1:2], in_=ssq[:, 0:NBLK],
                    op=mybir.AluOpType.add, axis=mybir.AxisListType.X)
                gst = scr.tile([P, 2], f32, tag="gst")
                nc.sync.dma_start(out=st_in[layer][:], in_=stats[:])
                nc.gpsimd.collective_compute(
                    "AllReduce", mybir.AluOpType.add,
                    ins=[st_in[layer][:]], outs=[st_out[layer][:]],
                    replica_groups=[list(range(NC))])
                nc.sync.dma_start(out=gst[:], in_=st_out[layer][:])

                mean = scr.tile([P, 1], f32, tag="mean")
                e2 = scr.tile([P, 1], f32, tag="e2")
                msq = scr.tile([P, 1], f32, tag="msq")
                var = scr.tile([P, 1], f32, tag="var")
                sd = scr.tile([P, 1], f32, tag="sd")
                isd = scr.tile([P, 1], f32, tag="isd")
                a_c = scr.tile([P, 1], f32, tag="a_c")
                mc = scr.tile([P, 1], f32, tag="mc")
                c_c = scr.tile([P, 1], f32, tag="c_c")
                inv_n = 1.0 / float(N_NODES)
                nc.scalar.activation(out=mean[:], in_=gst[:, 0:1],
                                     func=mybir.ActivationFunctionType.Copy, scale=inv_n)
                nc.scalar.activation(out=e2[:], in_=gst[:, 1:2],
                                     func=mybir.ActivationFunctionType.Copy, scale=inv_n)
                nc.scalar.square(out=msq[:], in_=mean[:])
                nc.vector.tensor_sub(out=var[:], in0=e2[:], in1=msq[:])
                nc.scalar.activation(out=sd[:], in_=var[:],
                                     func=mybir.ActivationFunctionType.Sqrt,
                                     bias=eps_t[:])
                nc.vector.reciprocal(out=isd[:], in_=sd[:])
                nc.vector.tensor_mul(out=a_c[:], in0=gamma[:, :], in1=isd[:])
                nc.vector.tensor_mul(out=mc[:], in0=mean[:], in1=a_c[:])
                nc.vector.tensor_sub(out=c_c[:], in0=beta[:, :], in1=mc[:])

                # ---- phase C: affines first (fills engine queues), then
                # transpose/copy/DMA pipelined across blocks ----
                if layer == 0:
                    for b in range(NBLK):
                        nb = min(P, NPC - b * P)
                        sl = slice(b * P, b * P + nb)
                        nc.scalar.activation(
                            out=hT0[:, sl], in_=hpre[:, sl],
                            func=mybir.ActivationFunctionType.Relu,
                            scale=a_c[:], bias=c_c[:])
                else:
                    for b in range(NBLK):
                        nb = min(P, NPC - b * P)
                        sl = slice(b * P, b * P + nb)
                        cb = bass.AP(c_c.tensor, c_c[:].offset,
                                     [c_c[:].ap[0], [0, nb]])
                        nc.vector.scalar_tensor_tensor(
                            out=hpre[:, sl], in0=hpre[:, sl], scalar=a_c[:],
                            in1=cb, op0=mybir.AluOpType.mult,
                            op1=mybir.AluOpType.add)
                for b in range(NBLK):
                    nb = min(P, NPC - b * P)
                    sl = slice(b * P, b * P + nb)
                    if layer == 0:
                        trb = psB.tile([P, P], f16, tag="tb")
                        nc.tensor.transpose(out=trb[0:nb, :], in_=hT0[:, sl],
                                            identity=idT16[:])
                        h0n = scr.tile([P, P], f16, tag="h0n")
                        nc.scalar.copy(out=h0n[0:nb, :], in_=trb[0:nb, :])
                        nc.sync.dma_start(out=h0_own[sl, :], in_=h0n[0:nb, :])
                        if b == RSPLIT // P - 1:
                            nc.gpsimd.collective_compute(
                                "AllGather", mybir.AluOpType.bypass,
                                ins=[h0_own[0:RSPLIT, :]], outs=[t0[:, :]],
                                replica_groups=[list(range(NC))])
                        elif b == NBLK - 1:
                            nc.gpsimd.collective_compute(
                                "AllGather", mybir.AluOpType.bypass,
                                ins=[h0_own[RSPLIT:NPC, :]], outs=[t1[:, :]],
                                replica_groups=[list(range(NC))])
                    else:
                        trb = psB.tile([P, P], f32, tag="tb")
                        nc.tensor.transpose(out=trb[0:nb, :], in_=hpre[:, sl],
                                            identity=idT32[:])
                        outn = scr.tile([P, P], f32, tag="outn")
                        nc.scalar.copy(out=outn[0:nb, :], in_=trb[0:nb, :])
                        nc.sync.dma_start(out=out_own[sl, :], in_=outn[0:nb, :])

                if layer == 0 and dbg_h0 is not None:
                    nc.sync.dma_start(out=dbg_h0[:, :], in_=h0_own[:, :])

    nc.compile()
    return nc


# ------------------------------------------------------------------- driver

_CACHE = {}


def kernel(x, edge_index, W_l0, b_l0, W_r0, gamma0, beta0,
           W_l1, b_l1, W_r1, gamma1, beta1):
    x = np.asarray(x, dtype=np.float32)
    edge_index = np.asarray(edge_index)

    sched = _build_schedule(edge_index)
    key = (len(sched["proc"]), sched["n_chunks"][0], sched["n_chunks"][1])
    if key not in _CACHE:
        _CACHE[key] = _build_program(sched)
    nc = _CACHE[key]

    x_pad = np.zeros((N_NODES, D), np.float16)
    x_pad[:, :IN_DIM] = x.astype(np.float16)
    xr = x_pad.reshape(NC, NPC, D)
    xs0 = np.ascontiguousarray(xr[:, :RSPLIT, :].reshape(SEG_ROWS[0], D))
    xs1 = np.ascontiguousarray(xr[:, RSPLIT:, :].reshape(SEG_ROWS[1], D))

    def pad_w(w):
        out = np.zeros((P, P), np.float16)
        out[: w.shape[0], : w.shape[1]] = np.asarray(w, dtype=np.float16)
        return out

    w_all = np.concatenate(
        [pad_w(W_l0), pad_w(W_r0), pad_w(W_l1), pad_w(W_r1)], axis=1)
    gb = np.stack([
        np.asarray(gamma0, np.float32), np.asarray(beta0, np.float32),
        np.asarray(gamma1, np.float32), np.asarray(beta1, np.float32)], axis=1)
    iota128 = np.tile(np.arange(P, dtype=np.float16)[None, :], (P, 1))
    ident = np.eye(P, dtype=np.float32)

    in_maps = []
    for c in range(NC):
        xT = np.zeros((P, NPC), np.float16)
        xT[:IN_DIM, :] = x[c * NPC : (c + 1) * NPC, :].T.astype(np.float16)
        in_maps.append(dict(
            xs0=xs0,
            xs1=xs1,
            idx_lo=sched["idx_lo"][c],
            idx_hi=sched["idx_hi"][c],
            dstloc=sched["dstloc"][c],
            xT_own=xT,
            iota128=iota128,
            id16=ident.astype(np.float16),
            id32=ident,
            w_all=w_all,
            gb=gb.astype(np.float32),
            invc_in=sched["invc"][c],
        ))

    res = run_bass_kernel_spmd(
        nc, in_maps, list(range(NC)),
        trace=bool(os.environ.get("KERNEL_TRACE")))
    global LAST_RESULT
    LAST_RESULT = res
    out = np.concatenate([res.results[c]["out_own"] for c in range(NC)], axis=0)
    return out.astype(np.float32)


LAST_RESULT = None
